# revision 13
# baseline (speedup 1.0000x reference)
"""Trainium2 Bass kernel for a GPT-style transformer block (B=2, T=2048, C=768, NH=12).

Sharding: 8 cores = 2 groups of 4 (one batch each); 512 tokens per core for
every phase. The only collective is a group-local AllGather of k (feature-major
pairs) and v (token-major, per-token all-head rows with a fused ones column for
the softmax denominator). Causality across chunks is enforced by zero-scaling
the gathered V with a per-core chunk mask (kills numerator and denominator
exactly); the in-chunk diagonal uses locally kept k/v tiles and a triangular
0/1 mask. Matmuls run in float32r (TF32-like, 1 cyc/row at N>=256); softmax is
max-free (scores are O(1)-bounded): exp(s/8) with the denominator from the
augmented-V matmul.
"""

import numpy as np

import concourse.bacc as bacc
import concourse.bass as bass
import concourse.tile as tile
from concourse import mybir
from concourse.bass_utils import run_bass_kernel_spmd
from concourse.masks import make_identity

f32 = mybir.dt.float32
f32r = mybir.dt.float32r
AF = mybir.ActivationFunctionType
OP = mybir.AluOpType

B, T, C, NH, HD = 2, 2048, 768, 12, 64
EPS = 1e-5
N_CORES, GROUP = 8, 4
TOK = 512                      # tokens per core
KC = C // 128                  # 6 feature chunks
FF = 4 * C                     # 3072
MT = FF // 128                 # 24 ff chunks
VROW = HD + 1                  # v row with ones column
HROW = NH * VROW               # all-head v row: 780 elements per token
KBLK = NH * HD * TOK           # gathered k block per core
VBLK = TOK * HROW              # gathered v block per core
TOT = KBLK + VBLK
SCALE = 1.0 / np.sqrt(HD)

_CACHE = {}


def _ap(handle, offset, pattern):
    return bass.AP(tensor=handle, offset=offset, ap=[list(p) for p in pattern])


def _build():
    nc = bacc.Bacc("TRN2", target_bir_lowering=False, debug=False,
                   num_devices=N_CORES)

    x_d = nc.dram_tensor("x", [TOK, C], f32, kind="ExternalInput")
    wqkv_d = nc.dram_tensor("wqkv", [C, 3 * C], f32r, kind="ExternalInput")
    bqkv_d = nc.dram_tensor("bqkv", [3 * C], f32, kind="ExternalInput")
    apw_d = nc.dram_tensor("apw", [C, C], f32r, kind="ExternalInput")
    apb_d = nc.dram_tensor("apb", [C], f32, kind="ExternalInput")
    ln1s_d = nc.dram_tensor("ln1s", [C], f32, kind="ExternalInput")
    ln1b_d = nc.dram_tensor("ln1b", [C], f32, kind="ExternalInput")
    ln2s_d = nc.dram_tensor("ln2s", [C], f32, kind="ExternalInput")
    ln2b_d = nc.dram_tensor("ln2b", [C], f32, kind="ExternalInput")
    fcw_d = nc.dram_tensor("fcw", [C, FF], f32r, kind="ExternalInput")
    fcb_d = nc.dram_tensor("fcb", [FF], f32, kind="ExternalInput")
    pjw_d = nc.dram_tensor("pjw", [FF, C], f32r, kind="ExternalInput")
    pjb_d = nc.dram_tensor("pjb", [C], f32, kind="ExternalInput")
    zmask_d = nc.dram_tensor("zmask", [GROUP], f32, kind="ExternalInput")
    out_d = nc.dram_tensor("out", [TOK, C], f32, kind="ExternalOutput")

    gin_d = nc.dram_tensor("g_in", [TOT], f32r, kind="Internal")
    gout_d = nc.dram_tensor("g_out", [GROUP, TOT], f32r, kind="Internal")

    groups = [list(range(g * GROUP, (g + 1) * GROUP)) for g in range(2)]

    with tile.TileContext(nc) as tc:
        with (
            tc.tile_pool(name="const", bufs=1) as cst,
            tc.tile_pool(name="resid", bufs=1) as res_pool,
            tc.tile_pool(name="acts", bufs=1) as act_pool,
        ):
            # ---------------- constants ----------------
            ones_f = cst.tile([128, 128], f32)
            nc.vector.memset(ones_f[:], 1.0)
            ones_r = cst.tile([128, 128], f32r)
            nc.vector.tensor_copy(ones_r[:], ones_f[:])
            ones_row_f = cst.tile([2, TOK], f32)
            nc.vector.memset(ones_row_f[:], 1.0)
            ones_row_r = cst.tile([1, TOK], f32r)
            nc.vector.tensor_copy(ones_row_r[:], ones_row_f[0:1, :])
            ones12 = cst.tile([128, NH, 1], f32)
            nc.vector.memset(ones12[:], 1.0)
            eps_t = cst.tile([1, 1], f32)
            nc.vector.memset(eps_t[:], EPS)
            # causal 0/1 mask: keep where kv_p <= q_f
            m01_f = cst.tile([128, 128], f32)
            nc.vector.memset(m01_f[:], 1.0)
            nc.gpsimd.affine_select(
                out=m01_f[:], in_=m01_f[:], compare_op=OP.is_ge, fill=0.0,
                base=0, pattern=[[1, 128]], channel_multiplier=-1)
            m01 = cst.tile([128, 128], f32r)
            nc.vector.tensor_copy(m01[:], m01_f[:])
            ident = cst.tile([128, 128], f32)
            make_identity(nc, ident[:])

            sc1 = cst.tile([1, C], f32r)
            nc.sync.dma_start(sc1[:], ln1s_d.ap().bitcast(f32r)[None, :])
            sb1 = cst.tile([1, C], f32r)
            nc.sync.dma_start(sb1[:], ln1b_d.ap().bitcast(f32r)[None, :])
            sc2 = cst.tile([1, C], f32r)
            nc.sync.dma_start(sc2[:], ln2s_d.ap().bitcast(f32r)[None, :])
            sb2 = cst.tile([1, C], f32r)
            nc.sync.dma_start(sb2[:], ln2b_d.ap().bitcast(f32r)[None, :])
            bqk = cst.tile([128, 2 * C // 128], f32)
            nc.sync.dma_start(
                bqk[:], bqkv_d.ap()[0:2 * C].rearrange("(a p) -> p a", p=128))
            bv_row = cst.tile([1, 3 * C], f32r)
            nc.sync.dma_start(bv_row[:], bqkv_d.ap().bitcast(f32r)[None, :])
            apb = cst.tile([128, KC], f32)
            nc.sync.dma_start(
                apb[:], apb_d.ap().rearrange("(a p) -> p a", p=128))
            fcb = cst.tile([128, MT], f32)
            nc.sync.dma_start(
                fcb[:], fcb_d.ap().rearrange("(a p) -> p a", p=128))
            pjb = cst.tile([128, KC], f32)
            nc.sync.dma_start(
                pjb[:], pjb_d.ap().rearrange("(a p) -> p a", p=128))
            zm = cst.tile([128, GROUP], f32)
            nc.sync.dma_start(zm[:], _ap(zmask_d, 0, [[0, 128], [1, GROUP]]))

            # ---------------- residual stream xT ----------------
            # token-major load (contiguous lines), PE-transpose to feature-major
            xT = [res_pool.tile([128, TOK], f32, name=f"xT{k}")
                  for k in range(KC)]
            with (
                tc.tile_pool(name="xtm", bufs=2) as xtm_pool,
                tc.tile_pool(name="tps", bufs=4, space="PSUM") as t_ps,
            ):
                for tt in range(4):
                    x_tm = xtm_pool.tile([128, C], f32, name="x_tm")
                    nc.sync.dma_start(
                        x_tm[:], x_d.ap()[tt * 128:(tt + 1) * 128, :])
                    for k in range(KC):
                        tp = t_ps.tile([128, 128], f32, name="tp")
                        nc.tensor.transpose(
                            tp[:], x_tm[:, k * 128:(k + 1) * 128], ident[:])
                        nc.vector.tensor_copy(
                            xT[k][:, tt * 128:(tt + 1) * 128], tp[:])

            # ---------------- LayerNorm helper ----------------
            def layer_norm(src, sc_row, sb_row, dst_pool, tag):
                dst = [dst_pool.tile([128, TOK], f32r, name=f"h{tag}{k}")
                       for k in range(KC)]
                with (
                    tc.tile_pool(name=f"lnps{tag}", bufs=1, space="PSUM") as lps,
                    tc.tile_pool(name=f"lnab{tag}", bufs=2, space="PSUM") as aps,
                    tc.tile_pool(name=f"lnsb{tag}", bufs=3) as lsb,
                ):
                    sum_x = lps.tile([1, TOK], f32, name=f"sumx{tag}")
                    sum_sq = lps.tile([1, TOK], f32, name=f"sumsq{tag}")
                    for k in range(KC):
                        sq = lsb.tile([128, TOK], f32r, name=f"sq{tag}")
                        nc.scalar.activation(sq[:], src[k][:], AF.Square)
                        nc.tensor.matmul(sum_x[:], ones_f[:, 0:1], src[k][:],
                                         start=(k == 0), stop=(k == KC - 1))
                        nc.tensor.matmul(sum_sq[:], ones_r[:, 0:1], sq[:],
                                         start=(k == 0), stop=(k == KC - 1))
                    mu = lsb.tile([1, TOK], f32, name=f"mu{tag}")
                    nc.scalar.activation(mu[:], sum_x[:], AF.Copy,
                                         bias=0.0, scale=1.0 / C)
                    ex2 = lsb.tile([1, TOK], f32, name=f"ex2{tag}")
                    nc.scalar.activation(ex2[:], sum_sq[:], AF.Copy,
                                         bias=0.0, scale=1.0 / C)
                    var = lsb.tile([1, TOK], f32, name=f"var{tag}")
                    nc.vector.tensor_mul(var[:], mu[:], mu[:])
                    nc.vector.tensor_sub(var[:], ex2[:], var[:])
                    sd = lsb.tile([1, TOK], f32, name=f"sd{tag}")
                    nc.scalar.activation(sd[:], var[:], AF.Sqrt,
                                         bias=eps_t[:], scale=1.0)
                    rstd = lsb.tile([1, TOK], f32r, name=f"rstd{tag}")
                    with nc.allow_low_precision(reason="fp32r rstd for PE"):
                        nc.vector.reciprocal(rstd[:], sd[:])
                    nmr = lsb.tile([1, TOK], f32r, name=f"nmr{tag}")
                    nc.vector.scalar_tensor_tensor(
                        out=nmr[:], in0=mu[:], scalar=-1.0, in1=rstd[:],
                        op0=OP.mult, op1=OP.mult)
                    for k in range(KC):
                        a_ps = aps.tile([128, TOK], f32, name=f"aps{tag}")
                        b_ps = aps.tile([128, TOK], f32, name=f"bps{tag}")
                        sl = slice(k * 128, (k + 1) * 128)
                        nc.tensor.matmul(a_ps[:], sc_row[0:1, sl], rstd[:],
                                         start=True, stop=True)
                        nc.tensor.matmul(b_ps[:], sc_row[0:1, sl], nmr[:],
                                         start=True, stop=False)
                        nc.tensor.matmul(b_ps[:], sb_row[0:1, sl],
                                         ones_row_r[:], start=False, stop=True)
                        tmp = lsb.tile([128, TOK], f32, name=f"tmp{tag}")
                        nc.vector.tensor_mul(tmp[:], src[k][:], a_ps[:])
                        nc.vector.tensor_add(dst[k][:], tmp[:], b_ps[:])
                return dst

            # ---------------- LN1 + QKV projection ----------------
            qa_stack = tile.ExitStack() if False else None
            import contextlib
            qa_ctx = contextlib.ExitStack()
            qk_keep = qa_ctx.enter_context(tc.tile_pool(name="qkkeep", bufs=1))
            vt_pool = qa_ctx.enter_context(tc.tile_pool(name="vtp", bufs=1))
            ctxT_pool = qa_ctx.enter_context(tc.tile_pool(name="ctxTp", bufs=1))
            hln_ctx = contextlib.ExitStack()
            hln_pool = hln_ctx.enter_context(tc.tile_pool(name="hlnp", bufs=1))
            hln = layer_norm(xT, sc1, sb1, hln_pool, "1")

            q_pair = [qk_keep.tile([128, TOK], f32r, name=f"qp{i}")
                      for i in range(6)]
            k_pair = [qk_keep.tile([128, TOK], f32r, name=f"kp{i}")
                      for i in range(6)]
            v_t = [vt_pool.tile([128, NH, VROW], f32r, name=f"vt{tt}")
                   for tt in range(4)]

            with (
                tc.tile_pool(name="wqkv", bufs=1) as wq_pool,
                tc.tile_pool(name="qkps", bufs=3, space="PSUM") as qk_ps,
                tc.tile_pool(name="vps", bufs=2, space="PSUM") as v_ps,
            ):
                wq = [wq_pool.tile([128, 3 * C], f32r, name=f"wq{k}")
                      for k in range(KC)]
                for k in range(KC):
                    nc.sync.dma_start(
                        wq[k][:],
                        _ap(wqkv_d, k * 128 * 3 * C, [[3 * C, 128], [1, 3 * C]]))
                # k pairs first (heads 2i,2i+1 at partitions 0/64)
                for i in range(6):
                    ps = qk_ps.tile([128, TOK], f32, name="qkp")
                    col = C + i * 128
                    for k in range(KC):
                        nc.tensor.matmul(
                            ps[:], wq[k][:, col:col + 128], hln[k][:],
                            start=(k == 0), stop=(k == KC - 1))
                    nc.scalar.activation(k_pair[i][:], ps[:], AF.Identity,
                                         bias=bqk[:, 6 + i:7 + i], scale=1.0)
                    nc.sync.dma_start(
                        _ap(gin_d, i * 128 * TOK, [[TOK, 128], [1, TOK]]),
                        k_pair[i][:])
                # v token-major, all heads + ones column, bias folded
                for tt in range(4):
                    vp = v_ps.tile([128, C], f32, name="vp")
                    nc.tensor.matmul(vp[:, 0:512], ones_r[0:1, :],
                                     bv_row[0:1, 2 * C:2 * C + 512],
                                     start=True, stop=False)
                    nc.tensor.matmul(vp[:, 512:768], ones_r[0:1, :],
                                     bv_row[0:1, 2 * C + 512:3 * C],
                                     start=True, stop=False)
                    tsl = slice(tt * 128, (tt + 1) * 128)
                    for k in range(KC):
                        nc.tensor.matmul(
                            vp[:, 0:512], hln[k][:, tsl],
                            wq[k][:, 2 * C:2 * C + 512],
                            start=False, stop=(k == KC - 1))
                        nc.tensor.matmul(
                            vp[:, 512:768], hln[k][:, tsl],
                            wq[k][:, 2 * C + 512:3 * C],
                            start=False, stop=(k == KC - 1))
                    nc.vector.tensor_copy(
                        v_t[tt][:, :, 0:HD],
                        vp[:].rearrange("p (h e) -> p h e", e=HD))
                    nc.vector.tensor_copy(v_t[tt][:, :, HD:VROW], ones12[:])
                    nc.sync.dma_start(
                        _ap(gin_d, KBLK + tt * 128 * HROW,
                            [[HROW, 128], [1, HROW]]),
                        v_t[tt][:])
                # gather k,v while q computes below
                nc.gpsimd.collective_compute(
                    "AllGather", OP.bypass, replica_groups=groups,
                    ins=[gin_d.ap().opt()], outs=[gout_d.ap().opt()])
                # q pairs (stay local)
                for i in range(6):
                    ps = qk_ps.tile([128, TOK], f32, name="qkp")
                    for k in range(KC):
                        nc.tensor.matmul(
                            ps[:], wq[k][:, i * 128:(i + 1) * 128], hln[k][:],
                            start=(k == 0), stop=(k == KC - 1))
                    nc.scalar.activation(q_pair[i][:], ps[:], AF.Identity,
                                         bias=bqk[:, i:i + 1], scale=1.0)

            hln_ctx.close()

            # ---------------- attention (local q, gathered k/v) -------------
            ctxT = [ctxT_pool.tile([128, TOK], f32r, name=f"ctxT{k}")
                    for k in range(KC)]
            with (
                tc.tile_pool(name="atkg", bufs=3) as kg_pool,
                tc.tile_pool(name="atva", bufs=5) as va_pool,
                tc.tile_pool(name="ate", bufs=3) as e_pool,
                tc.tile_pool(name="atd", bufs=2) as d_pool,
                tc.tile_pool(name="atcs", bufs=2) as cs_pool,
                tc.tile_pool(name="stps", bufs=1, space="PSUM") as st_ps,
                tc.tile_pool(name="rbps", bufs=1, space="PSUM") as rb_ps,
                tc.tile_pool(name="ctxps", bufs=4, space="PSUM") as ctx_psp,
            ):
                for bat in range(3):  # 3 batches x 4 heads (2 pairs)
                    pairs = (2 * bat, 2 * bat + 1)
                    ctx_ps = {}
                    # local diagonal chunk: q == own tokens, triangular
                    for i in pairs:
                        for half in range(2):
                            h = 2 * i + half
                            hb = 64 * half
                            cp = ctx_psp.tile([VROW, TOK], f32, name="ctxp")
                            ctx_ps[h] = cp
                            sT = st_ps.tile([128, 2, TOK], f32, name="sT")
                            ep = e_pool.tile([128, 2, TOK], f32r, name="ep")
                            for kt in range(4):
                                q0 = kt * 128
                                nc.tensor.matmul(
                                    sT[:, kt % 2, q0:TOK],
                                    k_pair[i][hb:hb + 64,
                                              kt * 128:(kt + 1) * 128],
                                    q_pair[i][hb:hb + 64, q0:TOK],
                                    start=True, stop=True)
                                nc.scalar.activation(
                                    ep[:, kt % 2, q0:TOK],
                                    sT[:, kt % 2, q0:TOK],
                                    AF.Exp, bias=0.0, scale=SCALE)
                                nc.vector.tensor_mul(
                                    ep[:, kt % 2, q0:q0 + 128],
                                    ep[:, kt % 2, q0:q0 + 128], m01[:])
                                nc.tensor.matmul(
                                    cp[:, q0:TOK], v_t[kt][:, h, :],
                                    ep[:, kt % 2, q0:TOK],
                                    start=(kt == 0), stop=False)
                                if kt % 2 == 1 and kt < 3:
                                    sT = st_ps.tile([128, 2, TOK], f32,
                                                    name="sT")
                                    ep = e_pool.tile([128, 2, TOK], f32r,
                                                     name="ep")
                    # gathered chunks, causality via zm scaling of v
                    for c in range(GROUP):
                        va = []
                        for kt in range(4):
                            vat = va_pool.tile([128, NH, VROW], f32r,
                                               name="vat")
                            nc.sync.dma_start(
                                vat[:],
                                _ap(gout_d,
                                    c * TOT + KBLK + kt * 128 * HROW,
                                    [[HROW, 128], [1, HROW]]))
                            nc.vector.tensor_scalar_mul(
                                vat[:], vat[:], zm[:, c:c + 1])
                            va.append(vat)
                        for i in pairs:
                            kg = kg_pool.tile([128, TOK], f32r, name="kg")
                            nc.sync.dma_start(
                                kg[:],
                                _ap(gout_d, c * TOT + i * 128 * TOK,
                                    [[TOK, 128], [1, TOK]]))
                            for half in range(2):
                                h = 2 * i + half
                                hb = 64 * half
                                cp = ctx_ps[h]
                                for hkt in range(2):
                                    sT = st_ps.tile([128, 2, TOK], f32,
                                                    name="sT")
                                    ep = e_pool.tile([128, 2, TOK], f32r,
                                                     name="ep")
                                    for sub in range(2):
                                        kt = 2 * hkt + sub
                                        nc.tensor.matmul(
                                            sT[:, sub, :],
                                            kg[hb:hb + 64,
                                               kt * 128:(kt + 1) * 128],
                                            q_pair[i][hb:hb + 64, :],
                                            start=True, stop=True)
                                    nc.scalar.activation(
                                        ep[:, :, :], sT[:, :, :],
                                        AF.Exp, bias=0.0, scale=SCALE)
                                    for sub in range(2):
                                        kt = 2 * hkt + sub
                                        nc.tensor.matmul(
                                            cp[:], va[kt][:, h, :],
                                            ep[:, sub, :],
                                            start=False,
                                            stop=(c == GROUP - 1
                                                  and kt == 3))
                    # normalize + write into feature-major ctxT
                    for i in pairs:
                        for half in range(2):
                            h = 2 * i + half
                            cp = ctx_ps[h]
                            den = d_pool.tile([VROW, TOK], f32r, name="den")
                            with nc.allow_low_precision(
                                    reason="fp32r softmax denom"):
                                nc.vector.reciprocal(den[64:65, :],
                                                     cp[64:65, :])
                            rb = rb_ps.tile([64, TOK], f32, name="rb")
                            nc.tensor.matmul(rb[:], ones_r[64:65, 0:64],
                                             den[64:65, :],
                                             start=True, stop=True)
                            rb_sb = cs_pool.tile([64, TOK], f32, name="rb_sb")
                            nc.scalar.copy(rb_sb[:], rb[:])
                            if half == 0:
                                nc.vector.tensor_mul(
                                    ctxT[i][0:64, :], cp[0:64, :], rb_sb[:])
                            else:
                                csb = cs_pool.tile([64, TOK], f32r,
                                                   name="csb")
                                nc.vector.tensor_mul(
                                    csb[:], cp[0:64, :], rb_sb[:])
                                nc.sync.dma_start(
                                    ctxT[i][64:128, :], csb[:])

            # ---------------- attn_proj + residual ----------------
            x1T = [res_pool.tile([128, TOK], f32, name=f"x1T{k}")
                   for k in range(KC)]
            with (
                tc.tile_pool(name="apwp", bufs=1) as apw_pool,
                tc.tile_pool(name="apps", bufs=3, space="PSUM") as ap_ps,
            ):
                apw = [apw_pool.tile([128, C], f32r, name=f"apw{k}")
                       for k in range(KC)]
                for k in range(KC):
                    nc.sync.dma_start(
                        apw[k][:], _ap(apw_d, k * 128 * C, [[C, 128], [1, C]]))
                for kc in range(KC):
                    ps = ap_ps.tile([128, TOK], f32, name="app")
                    for k in range(KC):
                        nc.tensor.matmul(
                            ps[:], apw[k][:, kc * 128:(kc + 1) * 128],
                            ctxT[k][:], start=(k == 0), stop=(k == KC - 1))
                    nc.vector.scalar_tensor_tensor(
                        out=x1T[kc][:], in0=ps[:], scalar=apb[:, kc:kc + 1],
                        in1=xT[kc][:], op0=OP.add, op1=OP.add)

            qa_ctx.close()

            # ---------------- LN2 + MLP ----------------
            h2_ctx = contextlib.ExitStack()
            h2_pool = h2_ctx.enter_context(tc.tile_pool(name="h2p", bufs=1))
            h2 = layer_norm(x1T, sc2, sb2, h2_pool, "2")

            o_sb = [res_pool.tile([128, TOK], f32, name=f"o_sb{kc}")
                    for kc in range(KC)]
            with (
                tc.tile_pool(name="fcwp", bufs=1) as fcw_pool,
                tc.tile_pool(name="pjwp", bufs=3) as pjw_pool,
                tc.tile_pool(name="gsb", bufs=3) as g_pool,
                tc.tile_pool(name="fcps", bufs=2, space="PSUM") as fc_ps,
                tc.tile_pool(name="pops", bufs=1, space="PSUM") as po_ps,
            ):
                fcw = [fcw_pool.tile([128, FF], f32r, name=f"fcw{k}")
                       for k in range(KC)]
                for k in range(KC):
                    nc.sync.dma_start(
                        fcw[k][:],
                        _ap(fcw_d, k * 128 * FF, [[FF, 128], [1, FF]]))
                pos = [po_ps.tile([128, TOK], f32, name=f"po{kc}")
                       for kc in range(KC)]
                for m in range(MT):
                    gp = fc_ps.tile([128, TOK], f32, name="gp")
                    for k in range(KC):
                        nc.tensor.matmul(
                            gp[:], fcw[k][:, m * 128:(m + 1) * 128], h2[k][:],
                            start=(k == 0), stop=(k == KC - 1))
                    g = g_pool.tile([128, TOK], f32r, name="g")
                    nc.scalar.activation(g[:], gp[:], AF.Gelu_apprx_tanh,
                                         bias=fcb[:, m:m + 1], scale=1.0)
                    pw = pjw_pool.tile([128, C], f32r, name="pw")
                    nc.sync.dma_start(
                        pw[:], _ap(pjw_d, m * 128 * C, [[C, 128], [1, C]]))
                    for kc in range(KC):
                        nc.tensor.matmul(
                            pos[kc][:], pw[:, kc * 128:(kc + 1) * 128], g[:],
                            start=(m == 0), stop=(m == MT - 1))
                for kc in range(KC):
                    nc.vector.scalar_tensor_tensor(
                        out=o_sb[kc][:], in0=pos[kc][:],
                        scalar=pjb[:, kc:kc + 1],
                        in1=x1T[kc][:], op0=OP.add, op1=OP.add)
            h2_ctx.close()
            # transpose back to token-major and store contiguously
            with (
                tc.tile_pool(name="otm", bufs=2) as otm_pool,
                tc.tile_pool(name="ops", bufs=4, space="PSUM") as o_ps,
            ):
                for tt in range(4):
                    o_tm = otm_pool.tile([128, C], f32, name="o_tm")
                    for kc in range(KC):
                        tp2 = o_ps.tile([128, 128], f32, name="tp2")
                        nc.tensor.transpose(
                            tp2[:], o_sb[kc][:, tt * 128:(tt + 1) * 128],
                            ident[:])
                        nc.vector.tensor_copy(
                            o_tm[:, kc * 128:(kc + 1) * 128], tp2[:])
                    nc.sync.dma_start(
                        out_d.ap()[tt * 128:(tt + 1) * 128, :], o_tm[:])

    nc.compile()
    return nc


def kernel(x, mask, ln1_scale, ln1_bias, wqkv, bqkv, attn_proj_w, attn_proj_b,
           ln2_scale, ln2_bias, fc_w, fc_b, proj_w, proj_b):
    x = np.asarray(x, dtype=np.float32)
    if "nc" not in _CACHE:
        _CACHE["nc"] = _build()
    nc = _CACHE["nc"]

    shared = {
        "wqkv": np.ascontiguousarray(
            np.asarray(wqkv, np.float32).reshape(C, 3 * C)),
        "bqkv": np.ascontiguousarray(
            np.asarray(bqkv, np.float32).reshape(3 * C)),
        "apw": np.ascontiguousarray(np.asarray(attn_proj_w, np.float32)),
        "apb": np.ascontiguousarray(np.asarray(attn_proj_b, np.float32)),
        "ln1s": np.ascontiguousarray(np.asarray(ln1_scale, np.float32)),
        "ln1b": np.ascontiguousarray(np.asarray(ln1_bias, np.float32)),
        "ln2s": np.ascontiguousarray(np.asarray(ln2_scale, np.float32)),
        "ln2b": np.ascontiguousarray(np.asarray(ln2_bias, np.float32)),
        "fcw": np.ascontiguousarray(np.asarray(fc_w, np.float32)),
        "fcb": np.ascontiguousarray(np.asarray(fc_b, np.float32)),
        "pjw": np.ascontiguousarray(np.asarray(proj_w, np.float32)),
        "pjb": np.ascontiguousarray(np.asarray(proj_b, np.float32)),
    }
    in_maps = []
    for core in range(N_CORES):
        b, r = divmod(core, GROUP)
        m = dict(shared)
        m["x"] = np.ascontiguousarray(x[b, r * TOK:(r + 1) * TOK, :])
        m["zmask"] = (np.arange(GROUP) < r).astype(np.float32)
        in_maps.append(m)

    res = run_bass_kernel_spmd(nc, in_maps, list(range(N_CORES)))
    _CACHE["last_result"] = res
    out = np.empty((B, T, C), dtype=np.float32)
    for core in range(N_CORES):
        b, r = divmod(core, GROUP)
        out[b, r * TOK:(r + 1) * TOK, :] = res.results[core]["out"]
    return out


# revision 15
# speedup vs baseline: 1.4694x; 1.4694x over previous
"""Trainium2 Bass kernel for a GPT-style transformer block (B=2, T=2048, C=768, NH=12).

Sharding: 8 cores = 2 groups of 4 (one batch each); 512 tokens per core for
every phase. The only collective is a group-local AllGather of k (feature-major
pairs) and v (token-major, per-token all-head rows with a fused ones column for
the softmax denominator). Causality across chunks is enforced by zero-scaling
the gathered V with a per-core chunk mask (kills numerator and denominator
exactly); the in-chunk diagonal uses locally kept k/v tiles and a triangular
0/1 mask. Matmuls run in float32r (TF32-like, 1 cyc/row at N>=256); softmax is
max-free (scores are O(1)-bounded): exp(s/8) with the denominator from the
augmented-V matmul.
"""

import numpy as np

import concourse.bacc as bacc
import concourse.bass as bass
import concourse.tile as tile
from concourse import mybir
from concourse.bass_utils import run_bass_kernel_spmd
from concourse.masks import make_identity

f32 = mybir.dt.float32
f32r = mybir.dt.float32r
bf16 = mybir.dt.bfloat16
AF = mybir.ActivationFunctionType
OP = mybir.AluOpType

B, T, C, NH, HD = 2, 2048, 768, 12, 64
EPS = 1e-5
N_CORES, GROUP = 8, 4
TOK = 512                      # tokens per core
KC = C // 128                  # 6 feature chunks
FF = 4 * C                     # 3072
MT = FF // 128                 # 24 ff chunks
VROW = HD + 1                  # v row with ones column
HROW = NH * VROW               # all-head v row: 780 elements per token
KBLK = NH * HD * TOK           # gathered k block per core
VBLK = TOK * HROW              # gathered v block per core
TOT = KBLK + VBLK
SCALE = 1.0 / np.sqrt(HD)

_CACHE = {}


def _ap(handle, offset, pattern):
    return bass.AP(tensor=handle, offset=offset, ap=[list(p) for p in pattern])


def _build():
    nc = bacc.Bacc("TRN2", target_bir_lowering=False, debug=False,
                   num_devices=N_CORES)

    x_d = nc.dram_tensor("x", [TOK, C], f32, kind="ExternalInput")
    wqkv_d = nc.dram_tensor("wqkv", [C, 3 * C], f32r, kind="ExternalInput")
    bqkv_d = nc.dram_tensor("bqkv", [3 * C], f32, kind="ExternalInput")
    apw_d = nc.dram_tensor("apw", [C, C], f32r, kind="ExternalInput")
    apb_d = nc.dram_tensor("apb", [C], f32, kind="ExternalInput")
    ln1s_d = nc.dram_tensor("ln1s", [C], f32, kind="ExternalInput")
    ln1b_d = nc.dram_tensor("ln1b", [C], f32, kind="ExternalInput")
    ln2s_d = nc.dram_tensor("ln2s", [C], f32, kind="ExternalInput")
    ln2b_d = nc.dram_tensor("ln2b", [C], f32, kind="ExternalInput")
    fcw_d = nc.dram_tensor("fcw", [C, FF], f32r, kind="ExternalInput")
    fcb_d = nc.dram_tensor("fcb", [FF], f32, kind="ExternalInput")
    pjw_d = nc.dram_tensor("pjw", [FF, C], f32r, kind="ExternalInput")
    pjb_d = nc.dram_tensor("pjb", [C], f32, kind="ExternalInput")
    zmask_d = nc.dram_tensor("zmask", [GROUP], f32, kind="ExternalInput")
    out_d = nc.dram_tensor("out", [TOK, C], f32, kind="ExternalOutput")

    gin_d = nc.dram_tensor("g_in", [TOT], bf16, kind="Internal")
    gout_d = nc.dram_tensor("g_out", [GROUP, TOT], bf16, kind="Internal")

    groups = [list(range(g * GROUP, (g + 1) * GROUP)) for g in range(2)]

    with tile.TileContext(nc) as tc:
        with (
            tc.tile_pool(name="const", bufs=1) as cst,
            tc.tile_pool(name="resid", bufs=1) as res_pool,
            tc.tile_pool(name="acts", bufs=1) as act_pool,
        ):
            # ---------------- constants ----------------
            ones_f = cst.tile([128, 128], f32)
            nc.vector.memset(ones_f[:], 1.0)
            ones_r = cst.tile([128, 128], f32r)
            nc.vector.tensor_copy(ones_r[:], ones_f[:])
            ones_row_f = cst.tile([2, TOK], f32)
            nc.vector.memset(ones_row_f[:], 1.0)
            ones_row_r = cst.tile([1, TOK], f32r)
            nc.vector.tensor_copy(ones_row_r[:], ones_row_f[0:1, :])
            ones12 = cst.tile([128, NH, 1], f32)
            nc.vector.memset(ones12[:], 1.0)
            eps_t = cst.tile([1, 1], f32)
            nc.vector.memset(eps_t[:], EPS)
            # causal 0/1 mask: keep where kv_p <= q_f
            m01_f = cst.tile([128, 128], f32)
            nc.vector.memset(m01_f[:], 1.0)
            nc.gpsimd.affine_select(
                out=m01_f[:], in_=m01_f[:], compare_op=OP.is_ge, fill=0.0,
                base=0, pattern=[[1, 128]], channel_multiplier=-1)
            m01 = cst.tile([128, 128], bf16)
            nc.vector.tensor_copy(m01[:], m01_f[:])
            ident = cst.tile([128, 128], f32)
            make_identity(nc, ident[:])

            sc1 = cst.tile([1, C], f32r)
            nc.sync.dma_start(sc1[:], ln1s_d.ap().bitcast(f32r)[None, :])
            sb1 = cst.tile([1, C], f32r)
            nc.sync.dma_start(sb1[:], ln1b_d.ap().bitcast(f32r)[None, :])
            sc2 = cst.tile([1, C], f32r)
            nc.sync.dma_start(sc2[:], ln2s_d.ap().bitcast(f32r)[None, :])
            sb2 = cst.tile([1, C], f32r)
            nc.sync.dma_start(sb2[:], ln2b_d.ap().bitcast(f32r)[None, :])
            bqk = cst.tile([128, 2 * C // 128], f32)
            nc.sync.dma_start(
                bqk[:], bqkv_d.ap()[0:2 * C].rearrange("(a p) -> p a", p=128))
            bv_row = cst.tile([1, 3 * C], f32r)
            nc.sync.dma_start(bv_row[:], bqkv_d.ap().bitcast(f32r)[None, :])
            apb = cst.tile([128, KC], f32)
            nc.sync.dma_start(
                apb[:], apb_d.ap().rearrange("(a p) -> p a", p=128))
            fcb = cst.tile([128, MT], f32)
            nc.sync.dma_start(
                fcb[:], fcb_d.ap().rearrange("(a p) -> p a", p=128))
            pjb = cst.tile([128, KC], f32)
            nc.sync.dma_start(
                pjb[:], pjb_d.ap().rearrange("(a p) -> p a", p=128))
            zm = cst.tile([128, GROUP], f32)
            nc.sync.dma_start(zm[:], _ap(zmask_d, 0, [[0, 128], [1, GROUP]]))

            # ---------------- residual stream xT ----------------
            # token-major load (contiguous lines), PE-transpose to feature-major
            xT = [res_pool.tile([128, TOK], f32, name=f"xT{k}")
                  for k in range(KC)]
            with (
                tc.tile_pool(name="xtm", bufs=2) as xtm_pool,
                tc.tile_pool(name="tps", bufs=4, space="PSUM") as t_ps,
            ):
                for tt in range(4):
                    x_tm = xtm_pool.tile([128, C], f32, name="x_tm")
                    nc.sync.dma_start(
                        x_tm[:], x_d.ap()[tt * 128:(tt + 1) * 128, :])
                    for k in range(KC):
                        tp = t_ps.tile([128, 128], f32, name="tp")
                        nc.tensor.transpose(
                            tp[:], x_tm[:, k * 128:(k + 1) * 128], ident[:])
                        nc.vector.tensor_copy(
                            xT[k][:, tt * 128:(tt + 1) * 128], tp[:])

            # ---------------- LayerNorm helper ----------------
            def layer_norm(src, sc_row, sb_row, dst_pool, tag):
                dst = [dst_pool.tile([128, TOK], f32r, name=f"h{tag}{k}")
                       for k in range(KC)]
                with (
                    tc.tile_pool(name=f"lnps{tag}", bufs=1, space="PSUM") as lps,
                    tc.tile_pool(name=f"lnab{tag}", bufs=2, space="PSUM") as aps,
                    tc.tile_pool(name=f"lnsb{tag}", bufs=3) as lsb,
                ):
                    sum_x = lps.tile([1, TOK], f32, name=f"sumx{tag}")
                    sum_sq = lps.tile([1, TOK], f32, name=f"sumsq{tag}")
                    for k in range(KC):
                        sq = lsb.tile([128, TOK], f32r, name=f"sq{tag}")
                        nc.scalar.activation(sq[:], src[k][:], AF.Square)
                        nc.tensor.matmul(sum_x[:], ones_f[:, 0:1], src[k][:],
                                         start=(k == 0), stop=(k == KC - 1))
                        nc.tensor.matmul(sum_sq[:], ones_r[:, 0:1], sq[:],
                                         start=(k == 0), stop=(k == KC - 1))
                    mu = lsb.tile([1, TOK], f32, name=f"mu{tag}")
                    nc.scalar.activation(mu[:], sum_x[:], AF.Copy,
                                         bias=0.0, scale=1.0 / C)
                    ex2 = lsb.tile([1, TOK], f32, name=f"ex2{tag}")
                    nc.scalar.activation(ex2[:], sum_sq[:], AF.Copy,
                                         bias=0.0, scale=1.0 / C)
                    var = lsb.tile([1, TOK], f32, name=f"var{tag}")
                    nc.vector.tensor_mul(var[:], mu[:], mu[:])
                    nc.vector.tensor_sub(var[:], ex2[:], var[:])
                    sd = lsb.tile([1, TOK], f32, name=f"sd{tag}")
                    nc.scalar.activation(sd[:], var[:], AF.Sqrt,
                                         bias=eps_t[:], scale=1.0)
                    rstd = lsb.tile([1, TOK], f32r, name=f"rstd{tag}")
                    with nc.allow_low_precision(reason="fp32r rstd for PE"):
                        nc.vector.reciprocal(rstd[:], sd[:])
                    nmr = lsb.tile([1, TOK], f32r, name=f"nmr{tag}")
                    nc.vector.scalar_tensor_tensor(
                        out=nmr[:], in0=mu[:], scalar=-1.0, in1=rstd[:],
                        op0=OP.mult, op1=OP.mult)
                    for k in range(KC):
                        a_ps = aps.tile([128, TOK], f32, name=f"aps{tag}")
                        b_ps = aps.tile([128, TOK], f32, name=f"bps{tag}")
                        sl = slice(k * 128, (k + 1) * 128)
                        nc.tensor.matmul(a_ps[:], sc_row[0:1, sl], rstd[:],
                                         start=True, stop=True)
                        nc.tensor.matmul(b_ps[:], sc_row[0:1, sl], nmr[:],
                                         start=True, stop=False)
                        nc.tensor.matmul(b_ps[:], sb_row[0:1, sl],
                                         ones_row_r[:], start=False, stop=True)
                        tmp = lsb.tile([128, TOK], f32, name=f"tmp{tag}")
                        nc.vector.tensor_mul(tmp[:], src[k][:], a_ps[:])
                        nc.vector.tensor_add(dst[k][:], tmp[:], b_ps[:])
                return dst

            # ---------------- LN1 + QKV projection ----------------
            qa_stack = tile.ExitStack() if False else None
            import contextlib
            qa_ctx = contextlib.ExitStack()
            qk_keep = qa_ctx.enter_context(tc.tile_pool(name="qkkeep", bufs=1))
            vt_pool = qa_ctx.enter_context(tc.tile_pool(name="vtp", bufs=1))
            ctxT_pool = qa_ctx.enter_context(tc.tile_pool(name="ctxTp", bufs=1))
            hln_ctx = contextlib.ExitStack()
            hln_pool = hln_ctx.enter_context(tc.tile_pool(name="hlnp", bufs=1))
            hln = layer_norm(xT, sc1, sb1, hln_pool, "1")

            q_pair = [qk_keep.tile([128, TOK], bf16, name=f"qp{i}")
                      for i in range(6)]
            k_pair = [qk_keep.tile([128, TOK], bf16, name=f"kp{i}")
                      for i in range(6)]
            v_t = [vt_pool.tile([128, NH, VROW], bf16, name=f"vt{tt}")
                   for tt in range(4)]

            with (
                tc.tile_pool(name="wqkv", bufs=1) as wq_pool,
                tc.tile_pool(name="qkps", bufs=3, space="PSUM") as qk_ps,
                tc.tile_pool(name="vps", bufs=2, space="PSUM") as v_ps,
            ):
                wq = [wq_pool.tile([128, 3 * C], f32r, name=f"wq{k}")
                      for k in range(KC)]
                for k in range(KC):
                    nc.sync.dma_start(
                        wq[k][:],
                        _ap(wqkv_d, k * 128 * 3 * C, [[3 * C, 128], [1, 3 * C]]))
                # k pairs first (heads 2i,2i+1 at partitions 0/64)
                for i in range(6):
                    ps = qk_ps.tile([128, TOK], f32, name="qkp")
                    col = C + i * 128
                    for k in range(KC):
                        nc.tensor.matmul(
                            ps[:], wq[k][:, col:col + 128], hln[k][:],
                            start=(k == 0), stop=(k == KC - 1))
                    nc.scalar.activation(k_pair[i][:], ps[:], AF.Identity,
                                         bias=bqk[:, 6 + i:7 + i], scale=1.0)
                    nc.sync.dma_start(
                        _ap(gin_d, i * 128 * TOK, [[TOK, 128], [1, TOK]]),
                        k_pair[i][:])
                # v token-major, all heads + ones column, bias folded
                for tt in range(4):
                    vp = v_ps.tile([128, C], f32, name="vp")
                    nc.tensor.matmul(vp[:, 0:512], ones_r[0:1, :],
                                     bv_row[0:1, 2 * C:2 * C + 512],
                                     start=True, stop=False)
                    nc.tensor.matmul(vp[:, 512:768], ones_r[0:1, :],
                                     bv_row[0:1, 2 * C + 512:3 * C],
                                     start=True, stop=False)
                    tsl = slice(tt * 128, (tt + 1) * 128)
                    for k in range(KC):
                        nc.tensor.matmul(
                            vp[:, 0:512], hln[k][:, tsl],
                            wq[k][:, 2 * C:2 * C + 512],
                            start=False, stop=(k == KC - 1))
                        nc.tensor.matmul(
                            vp[:, 512:768], hln[k][:, tsl],
                            wq[k][:, 2 * C + 512:3 * C],
                            start=False, stop=(k == KC - 1))
                    nc.vector.tensor_copy(
                        v_t[tt][:, :, 0:HD],
                        vp[:].rearrange("p (h e) -> p h e", e=HD))
                    nc.vector.tensor_copy(v_t[tt][:, :, HD:VROW], ones12[:])
                    nc.sync.dma_start(
                        _ap(gin_d, KBLK + tt * 128 * HROW,
                            [[HROW, 128], [1, HROW]]),
                        v_t[tt][:])
                # gather k,v while q computes below
                nc.gpsimd.collective_compute(
                    "AllGather", OP.bypass, replica_groups=groups,
                    ins=[gin_d.ap().opt()], outs=[gout_d.ap().opt()])
                # q pairs (stay local)
                for i in range(6):
                    ps = qk_ps.tile([128, TOK], f32, name="qkp")
                    for k in range(KC):
                        nc.tensor.matmul(
                            ps[:], wq[k][:, i * 128:(i + 1) * 128], hln[k][:],
                            start=(k == 0), stop=(k == KC - 1))
                    nc.scalar.activation(q_pair[i][:], ps[:], AF.Identity,
                                         bias=bqk[:, i:i + 1], scale=1.0)

            hln_ctx.close()

            # ---------------- attention (local q, gathered k/v) -------------
            ctxT = [ctxT_pool.tile([128, TOK], f32r, name=f"ctxT{k}")
                    for k in range(KC)]
            with (
                tc.tile_pool(name="atkg", bufs=3) as kg_pool,
                tc.tile_pool(name="atva", bufs=5) as va_pool,
                tc.tile_pool(name="ate", bufs=4) as e_pool,
                tc.tile_pool(name="atd", bufs=2) as d_pool,
                tc.tile_pool(name="atcs", bufs=3) as cs_pool,
                tc.tile_pool(name="atacc", bufs=1) as acc_pool,
                tc.tile_pool(name="stps", bufs=2, space="PSUM") as st_ps,
                tc.tile_pool(name="pvps", bufs=2, space="PSUM") as pv_ps,
                tc.tile_pool(name="rbps", bufs=1, space="PSUM") as rb_ps,
            ):
                ctx_acc = [acc_pool.tile([VROW, TOK], f32, name=f"cacc{h}")
                           for h in range(NH)]
                # local diagonal chunk: q == own tokens, triangular mask
                for i in range(6):
                    for half in range(2):
                        h = 2 * i + half
                        hb = 64 * half
                        pv = pv_ps.tile([VROW, TOK], f32, name="pv")
                        for hkt in range(2):
                            sT = st_ps.tile([128, 2, TOK], f32, name="sT")
                            ep = e_pool.tile([128, 2, TOK], bf16, name="ep")
                            for sub in range(2):
                                kt = 2 * hkt + sub
                                q0 = kt * 128
                                nc.tensor.matmul(
                                    sT[:, sub, q0:TOK],
                                    k_pair[i][hb:hb + 64,
                                              kt * 128:(kt + 1) * 128],
                                    q_pair[i][hb:hb + 64, q0:TOK],
                                    start=True, stop=True)
                                nc.scalar.activation(
                                    ep[:, sub, q0:TOK], sT[:, sub, q0:TOK],
                                    AF.Exp, bias=0.0, scale=SCALE)
                                nc.vector.tensor_mul(
                                    ep[:, sub, q0:q0 + 128],
                                    ep[:, sub, q0:q0 + 128], m01[:])
                                nc.tensor.matmul(
                                    pv[:, q0:TOK], v_t[kt][:, h, :],
                                    ep[:, sub, q0:TOK],
                                    start=(kt == 0), stop=(kt == 3))
                        nc.vector.tensor_copy(ctx_acc[h][:], pv[:])
                # gathered chunks, causality via zm scaling of v
                for c in range(GROUP):
                    va = []
                    for kt in range(4):
                        vat = va_pool.tile([128, NH, VROW], bf16, name="vat")
                        nc.sync.dma_start(
                            vat[:],
                            _ap(gout_d, c * TOT + KBLK + kt * 128 * HROW,
                                [[HROW, 128], [1, HROW]]))
                        nc.vector.tensor_scalar_mul(
                            vat[:], vat[:], zm[:, c:c + 1])
                        va.append(vat)
                    for i in range(6):
                        kg = kg_pool.tile([128, TOK], bf16, name="kg")
                        nc.sync.dma_start(
                            kg[:],
                            _ap(gout_d, c * TOT + i * 128 * TOK,
                                [[TOK, 128], [1, TOK]]))
                        for half in range(2):
                            h = 2 * i + half
                            hb = 64 * half
                            pv = pv_ps.tile([VROW, TOK], f32, name="pv")
                            for hkt in range(2):
                                sT = st_ps.tile([128, 2, TOK], f32, name="sT")
                                ep = e_pool.tile([128, 2, TOK], bf16,
                                                 name="ep")
                                for sub in range(2):
                                    kt = 2 * hkt + sub
                                    nc.tensor.matmul(
                                        sT[:, sub, :],
                                        kg[hb:hb + 64,
                                           kt * 128:(kt + 1) * 128],
                                        q_pair[i][hb:hb + 64, :],
                                        start=True, stop=True)
                                nc.scalar.activation(
                                    ep[:, :, :], sT[:, :, :],
                                    AF.Exp, bias=0.0, scale=SCALE)
                                for sub in range(2):
                                    kt = 2 * hkt + sub
                                    nc.tensor.matmul(
                                        pv[:], va[kt][:, h, :], ep[:, sub, :],
                                        start=(kt == 0), stop=(kt == 3))
                            nc.vector.tensor_add(ctx_acc[h][:], ctx_acc[h][:],
                                                 pv[:])
                # normalize + write into feature-major ctxT
                for i in range(6):
                    for half in range(2):
                        h = 2 * i + half
                        acc = ctx_acc[h]
                        den = d_pool.tile([VROW, TOK], f32r, name="den")
                        with nc.allow_low_precision(
                                reason="fp32r softmax denom"):
                            nc.vector.reciprocal(den[64:65, :], acc[64:65, :])
                        rb = rb_ps.tile([64, TOK], f32, name="rb")
                        nc.tensor.matmul(rb[:], ones_r[64:65, 0:64],
                                         den[64:65, :], start=True, stop=True)
                        rb_sb = cs_pool.tile([64, TOK], f32, name="rb_sb")
                        nc.scalar.copy(rb_sb[:], rb[:])
                        if half == 0:
                            nc.vector.tensor_mul(
                                ctxT[i][0:64, :], acc[0:64, :], rb_sb[:])
                        else:
                            csb = cs_pool.tile([64, TOK], f32r, name="csb")
                            nc.vector.tensor_mul(
                                csb[:], acc[0:64, :], rb_sb[:])
                            nc.sync.dma_start(ctxT[i][64:128, :], csb[:])

            # ---------------- attn_proj + residual ----------------
            x1T = [res_pool.tile([128, TOK], f32, name=f"x1T{k}")
                   for k in range(KC)]
            with (
                tc.tile_pool(name="apwp", bufs=1) as apw_pool,
                tc.tile_pool(name="apps", bufs=3, space="PSUM") as ap_ps,
            ):
                apw = [apw_pool.tile([128, C], f32r, name=f"apw{k}")
                       for k in range(KC)]
                for k in range(KC):
                    nc.sync.dma_start(
                        apw[k][:], _ap(apw_d, k * 128 * C, [[C, 128], [1, C]]))
                for kc in range(KC):
                    ps = ap_ps.tile([128, TOK], f32, name="app")
                    for k in range(KC):
                        nc.tensor.matmul(
                            ps[:], apw[k][:, kc * 128:(kc + 1) * 128],
                            ctxT[k][:], start=(k == 0), stop=(k == KC - 1))
                    nc.vector.scalar_tensor_tensor(
                        out=x1T[kc][:], in0=ps[:], scalar=apb[:, kc:kc + 1],
                        in1=xT[kc][:], op0=OP.add, op1=OP.add)

            qa_ctx.close()

            # ---------------- LN2 + MLP ----------------
            h2_ctx = contextlib.ExitStack()
            h2_pool = h2_ctx.enter_context(tc.tile_pool(name="h2p", bufs=1))
            h2 = layer_norm(x1T, sc2, sb2, h2_pool, "2")

            o_sb = [res_pool.tile([128, TOK], f32, name=f"o_sb{kc}")
                    for kc in range(KC)]
            with (
                tc.tile_pool(name="fcwp", bufs=1) as fcw_pool,
                tc.tile_pool(name="pjwp", bufs=3) as pjw_pool,
                tc.tile_pool(name="gsb", bufs=3) as g_pool,
                tc.tile_pool(name="fcps", bufs=2, space="PSUM") as fc_ps,
                tc.tile_pool(name="pops", bufs=1, space="PSUM") as po_ps,
            ):
                fcw = [fcw_pool.tile([128, FF], f32r, name=f"fcw{k}")
                       for k in range(KC)]
                for k in range(KC):
                    nc.sync.dma_start(
                        fcw[k][:],
                        _ap(fcw_d, k * 128 * FF, [[FF, 128], [1, FF]]))
                pos = [po_ps.tile([128, TOK], f32, name=f"po{kc}")
                       for kc in range(KC)]
                for m in range(MT):
                    gp = fc_ps.tile([128, TOK], f32, name="gp")
                    for k in range(KC):
                        nc.tensor.matmul(
                            gp[:], fcw[k][:, m * 128:(m + 1) * 128], h2[k][:],
                            start=(k == 0), stop=(k == KC - 1))
                    g = g_pool.tile([128, TOK], f32r, name="g")
                    nc.scalar.activation(g[:], gp[:], AF.Gelu_apprx_tanh,
                                         bias=fcb[:, m:m + 1], scale=1.0)
                    pw = pjw_pool.tile([128, C], f32r, name="pw")
                    nc.sync.dma_start(
                        pw[:], _ap(pjw_d, m * 128 * C, [[C, 128], [1, C]]))
                    for kc in range(KC):
                        nc.tensor.matmul(
                            pos[kc][:], pw[:, kc * 128:(kc + 1) * 128], g[:],
                            start=(m == 0), stop=(m == MT - 1))
                for kc in range(KC):
                    nc.vector.scalar_tensor_tensor(
                        out=o_sb[kc][:], in0=pos[kc][:],
                        scalar=pjb[:, kc:kc + 1],
                        in1=x1T[kc][:], op0=OP.add, op1=OP.add)
            h2_ctx.close()
            # transpose back to token-major and store contiguously
            with (
                tc.tile_pool(name="otm", bufs=2) as otm_pool,
                tc.tile_pool(name="ops", bufs=4, space="PSUM") as o_ps,
            ):
                for tt in range(4):
                    o_tm = otm_pool.tile([128, C], f32, name="o_tm")
                    for kc in range(KC):
                        tp2 = o_ps.tile([128, 128], f32, name="tp2")
                        nc.tensor.transpose(
                            tp2[:], o_sb[kc][:, tt * 128:(tt + 1) * 128],
                            ident[:])
                        nc.vector.tensor_copy(
                            o_tm[:, kc * 128:(kc + 1) * 128], tp2[:])
                    nc.sync.dma_start(
                        out_d.ap()[tt * 128:(tt + 1) * 128, :], o_tm[:])

    nc.compile()
    return nc


def kernel(x, mask, ln1_scale, ln1_bias, wqkv, bqkv, attn_proj_w, attn_proj_b,
           ln2_scale, ln2_bias, fc_w, fc_b, proj_w, proj_b):
    x = np.asarray(x, dtype=np.float32)
    if "nc" not in _CACHE:
        _CACHE["nc"] = _build()
    nc = _CACHE["nc"]

    shared = {
        "wqkv": np.ascontiguousarray(
            np.asarray(wqkv, np.float32).reshape(C, 3 * C)),
        "bqkv": np.ascontiguousarray(
            np.asarray(bqkv, np.float32).reshape(3 * C)),
        "apw": np.ascontiguousarray(np.asarray(attn_proj_w, np.float32)),
        "apb": np.ascontiguousarray(np.asarray(attn_proj_b, np.float32)),
        "ln1s": np.ascontiguousarray(np.asarray(ln1_scale, np.float32)),
        "ln1b": np.ascontiguousarray(np.asarray(ln1_bias, np.float32)),
        "ln2s": np.ascontiguousarray(np.asarray(ln2_scale, np.float32)),
        "ln2b": np.ascontiguousarray(np.asarray(ln2_bias, np.float32)),
        "fcw": np.ascontiguousarray(np.asarray(fc_w, np.float32)),
        "fcb": np.ascontiguousarray(np.asarray(fc_b, np.float32)),
        "pjw": np.ascontiguousarray(np.asarray(proj_w, np.float32)),
        "pjb": np.ascontiguousarray(np.asarray(proj_b, np.float32)),
    }
    in_maps = []
    for core in range(N_CORES):
        b, r = divmod(core, GROUP)
        m = dict(shared)
        m["x"] = np.ascontiguousarray(x[b, r * TOK:(r + 1) * TOK, :])
        m["zmask"] = (np.arange(GROUP) < r).astype(np.float32)
        in_maps.append(m)

    res = run_bass_kernel_spmd(nc, in_maps, list(range(N_CORES)))
    _CACHE["last_result"] = res
    out = np.empty((B, T, C), dtype=np.float32)
    for core in range(N_CORES):
        b, r = divmod(core, GROUP)
        out[b, r * TOK:(r + 1) * TOK, :] = res.results[core]["out"]
    return out
